# revision 22
# baseline (speedup 1.0000x reference)
"""Causal self-attention with post-softmax decay mask — Trainium2 Bass kernel.

Problem shapes (hardcoded): B=2, T=2048, C=1024, H=16 heads, head_dim=64.
Sharding: 8 cores = (batch b = core//4) x (head group g = core%4, 4 heads each).
Each core computes QKV projection for its 4 heads from x[b], causal
flash-style attention with the decay mask, and a partial output projection
(its heads' 256 features x W_proj rows). Host sums the 4 partials per batch.

Key structure (v3 — transpose-free attention, groups of 4 q-tiles):
  The attention probabilities are computed directly in TRANSPOSED
  orientation, S^T[k, q] = (K Q^T), with k on partitions — so the PV
  contraction (over k) needs no XBAR transpose of P (the per-block
  transpose dispatch cost ~160ns x 544 blocks dominated v1), and the
  group-of-4 q-tile batching gives F=512 matmuls (per-matmul overhead
  dominated v2's F=256/group-2 variant).

  Per GROUP of 4 q-tiles (q columns 512g..512g+511), per head:
    - S^T k-blocks, kcd DESCENDING (jj=0 <-> kc=4g+3).  The 4 staircase
      blocks jj=0..3 are the q-tiles' diagonals: block jj only has valid
      q columns >= 128*(3-jj); the causal mask (-1e30) is an extra matmul
      negI^T @ maskA into each diagonal's 128-col region.  Head pairs
      pack into PE row halves (K=64, tile_position).
    - exp via ScalarE (scale=1/8) into SBUF pT [128, jj, tqi, 128].
    - Z row sums via PE: ones32^T @ pT per block (m=32, F<=512),
      accumulated into a 32-row band of one PSUM tile per head (4 heads
      = 4 distinct PE column groups; the bands cover the tile so the
      reciprocal reads no uninitialized rows).  Z uses the UNdecayed exp
      (the reference normalizes before the multiplicative decay).
    - decay: DVE/GpSimd tensor_tensor, ONE op for all full blocks via the
      contiguous Toeplitz constant ddSEQ4[k, a, r, q] = d(128(a+r)+q-k)
      (a = jj-3, r = tqi) + 3 small staircase ops.  d<0 positions are 0,
      doubling as causal zeroing.  Runs after the Z matmuls (in-place).
    - PV: F=512 matmuls over kc ascending (all 4 q-tiles at once; the
      staircase blocks shrink to F=384/256/128).
    - normalization: reciprocal_approx_fast of the banded Z tile,
      selector-matmul broadcast (selP, K=128) to head-halves, applied on
      the PSUM->SBUF copy (scalar_tensor_tensor).
  K/Q/V strips are produced per 512-t chunk from a double-buffered xT
  chunk tile; V^T strips are flipped to [t, feat] with one XBAR transpose
  per (chunk, 128-feat pass).  Projection per q-tile: 2 pair-accumulated
  F=512 matmuls per 512-feature half, one [128,1024] output DMA.
"""

import math
import sys

sys.path.insert(0, "/opt/trn_rl_repo")

import numpy as np
import ml_dtypes

B, T, C = 2, 2048, 1024
N_HEAD = 16
HD = 64
HEADS_PER_CORE = 4
N_CORES = 8
NQ = T // 128  # 16 q-tiles

BF16 = ml_dtypes.bfloat16


def _decay_values_np(n):
    """decay_values[i] = decay at distance i, faithful to reference (fp64)."""
    dl = 2048 - 16 + 1
    nums = np.linspace(0.0, 1.0, dl, dtype=np.float64)
    decay_values = 1.0 - np.power(nums, 1.0 / np.e)
    return np.concatenate([np.ones(15), decay_values])[:n]


def _decay_matrix_np(n):
    """tril decay matrix, faithful to reference.decay_weight_matrix (fp32)."""
    decay_values = _decay_values_np(n)
    idx = np.arange(n)[:, None] - np.arange(n)[None, :]
    mat = decay_values[np.clip(idx, 0, n - 1)]
    return np.where(idx >= 0, mat, 0.0).astype(np.float32)


def build_nc(T_=T):
    import concourse.bass as bass
    import concourse.bacc as bacc
    import concourse.mybir as mybir
    import concourse.tile as tile

    fp32 = mybir.dt.float32
    bf16 = mybir.dt.bfloat16
    Alu = mybir.AluOpType
    Act = mybir.ActivationFunctionType

    NQ_ = T_ // 128
    NG_ = NQ_ // 4  # q-tile groups of 4
    assert NQ_ % 4 == 0
    nc = bacc.Bacc("TRN2")

    xT = nc.declare_dram_parameter("xT", [C, T_], bf16, isOutput=False)
    # wqkv columns: [q01(128) q23(128) k01(128) k23(128) v0..v3(256)]
    wqkv = nc.declare_dram_parameter("wqkv", [C, 768], bf16, isOutput=False)
    # wp rows: h0 feats(64), h1, h2, h3
    wp = nc.declare_dram_parameter("wp", [256, C], bf16, isOutput=False)
    # Toeplitz decay arranged to match pT's (block, tqi, q) layout:
    # ddSEQ4[k, a, r, q] = d(128(a + r) + q - k), 0 where negative.
    NA = max(1, NQ_ - 3)
    ddSEQ4 = nc.declare_dram_parameter("ddSEQ4", [128, NA, 4, 128], bf16, isOutput=False)
    # causal tril mask applied on the DVE to the 4 diagonal sub-blocks
    # (post-exp, pre-Z): tril4[k, a, q] = 1 iff k <= q.
    tril4 = nc.declare_dram_parameter("tril4", [128, 4, 128], bf16, isOutput=False)
    # Z row-sum matmul weights (all ones, m=32 band)
    ones32 = nc.declare_dram_parameter("ones32", [128, 32], bf16, isOutput=False)
    # selector broadcast: selP[j, 128*pair + p] = 1 iff j = 64*pair + 32*(p//64)
    selP = nc.declare_dram_parameter("selP", [128, 256], bf16, isOutput=False)
    out = nc.declare_dram_parameter("out", [T_, C], bf16, isOutput=True)

    with tile.TileContext(nc) as tc:
        with (
            tc.tile_pool(name="const", bufs=1) as const_pool,
            tc.tile_pool(name="xt", bufs=4) as xt_pool,
            tc.tile_pool(name="qkvout", bufs=1) as qkv_pool,
            tc.tile_pool(name="p", bufs=1) as p_pool,
            tc.tile_pool(name="z", bufs=3) as z_pool,
            tc.tile_pool(name="outs", bufs=2) as out_pool,
            tc.tile_pool(name="ps_s", bufs=2, space="PSUM") as ps_s,
            tc.tile_pool(name="ps_w", bufs=2, space="PSUM") as ps_w_pool,
            tc.tile_pool(name="ps_y", bufs=1, space="PSUM") as ps_y_pool,
            tc.tile_pool(name="ps_z", bufs=1, space="PSUM") as ps_z_pool,
        ):
            # ---- constants ----
            wqkv_sb = const_pool.tile([128, 8, 768], bf16)
            wqkv_r = wqkv.rearrange("(kc p) m -> p kc m", p=128)
            for kc in range(8):
                nc.sync.dma_start(
                    out=wqkv_sb[:, kc : kc + 1, :], in_=wqkv_r[:, kc : kc + 1, :]
                )
            wp_sb = const_pool.tile([128, 2, C], bf16)
            nc.sync.dma_start(out=wp_sb, in_=wp.rearrange("(pr p) n -> p pr n", p=128))
            dd_sb = const_pool.tile([128, NA, 4, 128], bf16)
            nc.sync.dma_start(out=dd_sb, in_=ddSEQ4[:, :, :, :])
            tril4_sb = const_pool.tile([128, 4, 128], bf16)
            nc.sync.dma_start(out=tril4_sb, in_=tril4[:, :, :])
            ones32_sb = const_pool.tile([128, 32], bf16)
            nc.sync.dma_start(out=ones32_sb, in_=ones32[:, :])
            selP_sb = const_pool.tile([128, 256], bf16)
            nc.sync.dma_start(out=selP_sb, in_=selP[:, :])

            xT_r = xT.rearrange("(kc p) t -> p kc t", p=128)
            qt_sb = qkv_pool.tile([128, 2, T_], bf16)  # [feat(2x64), pair, q]
            kt_sb = qkv_pool.tile([128, 2, T_], bf16)
            vt_sb = qkv_pool.tile([128, 2, T_], bf16)  # v^T strips [feat pass, t]
            v_sb = qkv_pool.tile([128, T_ // 128, 256], bf16)  # [k-rows, kc, 4 heads]
            yt_sb = qkv_pool.tile([128, 2, T_], bf16)  # y^T strips per pair

            cp_flip = [0]

            def cp(dst, src):
                """PSUM->SBUF copies alternate Vector/Scalar."""
                e = nc.vector.tensor_copy if cp_flip[0] % 2 == 0 else nc.scalar.copy
                cp_flip[0] += 1
                e(out=dst, in_=src)

            # ---- phase 1: QKV strip production per 512-t chunk ----
            # emitted as individual work items (one strip pass each) so
            # they can be interleaved between S-piece bursts: the PE queue
            # is in-order, and these give it exp-independent work to chew
            # on while ScalarE catches up.
            def load_xt(c):
                t0 = 512 * c
                cw = min(512, T_ - t0)
                xt_c = xt_pool.tile([128, 8, 512], bf16, tag="xt")
                nc.sync.dma_start(
                    out=xt_c[:, :, 0:cw], in_=xT_r[:, :, t0 : t0 + cw]
                )
                return xt_c

            def strip_items(c, xt_c):
                t0 = 512 * c
                cw = min(512, T_ - t0)

                def qk_pass(which, dst, pair):
                    def run():
                        wcol = 256 * which + 128 * pair
                        ps = ps_w_pool.tile([128, 512], fp32, tag="w")
                        for kc in range(8):
                            nc.tensor.matmul(
                                ps[:, 0:cw],
                                lhsT=wqkv_sb[:, kc, wcol : wcol + 128],
                                rhs=xt_c[:, kc, 0:cw],
                                start=(kc == 0),
                                stop=(kc == 7),
                            )
                        cp(dst[:, pair, t0 : t0 + cw], ps[:, 0:cw])

                    return run

                def v_pass(p):
                    def run():
                        ps = ps_w_pool.tile([128, 512], fp32, tag="w")
                        for kc in range(8):
                            nc.tensor.matmul(
                                ps[:, 0:cw],
                                lhsT=wqkv_sb[:, kc, 512 + 128 * p : 640 + 128 * p],
                                rhs=xt_c[:, kc, 0:cw],
                                start=(kc == 0),
                                stop=(kc == 7),
                            )
                        cp(vt_sb[:, p, t0 : t0 + cw], ps[:, 0:cw])
                        nc.sync.dma_start_transpose(
                            out=v_sb[
                                :, 4 * c : 4 * c + cw // 128, 128 * p : 128 * p + 128
                            ],
                            in_=vt_sb[:, p, t0 : t0 + cw],
                        )

                    return run

                return [
                    qk_pass(1, kt_sb, 0),
                    qk_pass(1, kt_sb, 1),
                    qk_pass(0, qt_sb, 0),
                    qk_pass(0, qt_sb, 1),
                    v_pass(0),
                    v_pass(1),
                ]

            # ---- phase 2: S^T / exp / Z / decay per group of 4 q-tiles ----
            def st_group(g, work):
                nblk = 4 * g + 4  # k-blocks, jj descending: jj=0 <-> kc=4g+3
                npiece = nblk // 2
                pTs = []
                for head in range(4):
                    # [k, jj, tqi, q]
                    pT = p_pool.tile([128, NQ_, 4, 128], bf16, tag=f"p{head}")
                    pTs.append(pT)
                zps = ps_z_pool.tile([128, 512], fp32, tag="z")
                q0 = 512 * g
                for piece in range(npiece):
                    j0 = 2 * piece
                    for head in range(4):
                        pair, hin = head // 2, head % 2
                        prow = 64 * hin
                        pT = pTs[head]
                        pTf = pT[:, :, :, :].rearrange("p a b q -> p (a b q)")
                        ps_full = ps_s.tile([128, 1024], fp32, tag="s")
                        ps = ps_full
                        for jj in (j0, j0 + 1):
                            kc = 4 * g + 3 - jj
                            c0 = 512 * (jj - j0)
                            lo = 128 * max(0, 3 - jj)
                            nc.tensor.matmul(
                                ps[:, c0 + lo : c0 + 512],
                                lhsT=kt_sb[
                                    prow : prow + 64, pair, 128 * kc : 128 * kc + 128
                                ],
                                rhs=qt_sb[prow : prow + 64, pair, q0 + lo : q0 + 512],
                                start=True,
                                stop=True,
                                tile_position=(prow, 0),
                            )
                        if piece == 0:
                            nc.scalar.activation(
                                out=pTf[:, 384:512],
                                in_=ps[:, 384:512],
                                func=Act.Exp,
                                scale=0.125,
                            )
                            nc.scalar.activation(
                                out=pTf[:, 768:1024],
                                in_=ps[:, 768:1024],
                                func=Act.Exp,
                                scale=0.125,
                            )
                        elif piece == 1:
                            nc.scalar.activation(
                                out=pTf[:, 1152:2048],
                                in_=ps[:, 128:1024],
                                func=Act.Exp,
                                scale=0.125,
                            )
                        else:
                            nc.scalar.activation(
                                out=pTf[:, 512 * j0 : 512 * (j0 + 2)],
                                in_=ps[:, 0:1024],
                                func=Act.Exp,
                                scale=0.125,
                            )
                    # exp-independent PE filler between S-piece bursts
                    if len(work) > 2:
                        work.pop(0)()
                # causal tril zeroing of the 4 diagonal sub-blocks (flat
                # cols 384 + 384*jj .. +128), one strided DVE op per head
                for head in range(4):
                    eng = nc.gpsimd if head == 3 else nc.vector
                    pTf = pTs[head][:, :, :, :].rearrange("p a b q -> p (a b q)")
                    diag = pTf[:, 384:1920].rearrange("p (a q) -> p a q", q=384)[
                        :, :, 0:128
                    ]
                    eng.tensor_tensor(
                        out=diag, in0=diag, in1=tril4_sb, op=Alu.mult
                    )
                # Z row sums (undecayed), kc ascending so the full-width
                # block opens the accumulation group; head-inner so the 4
                # column-group bands run concurrently on the PE
                for jj in range(nblk - 1, -1, -1):
                    lo3 = max(0, 3 - jj)
                    for head in range(4):
                        nc.tensor.matmul(
                            zps[32 * head : 32 * head + 32, 128 * lo3 : 512],
                            lhsT=ones32_sb,
                            rhs=pTs[head][:, jj, lo3:4, :],
                            start=(jj == nblk - 1),
                            stop=(jj == 0),
                            tile_position=(0, 32 * head),
                            skip_group_check=True,
                        )
                # decay (in place, after the Z sums): one op for the full
                # blocks + three staircase ops
                for head in range(4):
                    eng = nc.gpsimd if head == 3 else nc.vector
                    pT = pTs[head]
                    hi = nblk
                    while hi > 3:
                        lo_j = max(3, hi - 4)
                        eng.tensor_tensor(
                            out=pT[:, lo_j:hi, :, :],
                            in0=pT[:, lo_j:hi, :, :],
                            in1=dd_sb[:, lo_j - 3 : hi - 3, :, :],
                            op=Alu.mult,
                        )
                        hi = lo_j
                    eng.tensor_tensor(
                        out=pT[:, 2, 1:4, :],
                        in0=pT[:, 2, 1:4, :],
                        in1=dd_sb[:, 0, 0:3, :],
                        op=Alu.mult,
                    )
                    eng.tensor_tensor(
                        out=pT[:, 1, 2:4, :],
                        in0=pT[:, 1, 2:4, :],
                        in1=dd_sb[:, 0, 0:2, :],
                        op=Alu.mult,
                    )
                    eng.tensor_tensor(
                        out=pT[:, 0, 3, :],
                        in0=pT[:, 0, 3, :],
                        in1=dd_sb[:, 0, 0, :],
                        op=Alu.mult,
                    )
                return pTs, zps

            # ---- phase 3: grouped PV + deferred softmax normalization ----
            def pv_norm_group(g, pTs, zps, work):
                nblk = 4 * g + 4
                # filler items sit in the PE queue at the decay-wait and
                # inter-pair points; all are drained before this group ends
                if work:
                    work.pop(0)()
                rz = z_pool.tile([128, 512], fp32, tag="rz")
                nc.vector.reciprocal_approx_fast(out=rz, in_=zps)
                rzb = z_pool.tile([128, 512], bf16, tag="rzb")
                nc.vector.tensor_copy(out=rzb, in_=rz)
                for pair in range(2):
                    if work:
                        work.pop(0)()
                    ps_yg = ps_y_pool.tile([128, 512], fp32, tag="y")
                    for kc in range(nblk):
                        jj = 4 * g + 3 - kc
                        lo3 = max(0, 3 - jj)
                        for hin in range(2):
                            head = 2 * pair + hin
                            prow = 64 * hin
                            nc.tensor.matmul(
                                ps_yg[prow : prow + 64, 128 * lo3 : 512],
                                lhsT=v_sb[:, kc, 64 * head : 64 * head + 64],
                                rhs=pTs[head][:, jj, lo3:4, :],
                                start=(kc == 0),
                                stop=(kc == nblk - 1),
                                tile_position=(0, prow),
                                skip_group_check=True,
                            )
                    mbc = ps_w_pool.tile([128, 512], fp32, tag="w")
                    nc.tensor.matmul(
                        mbc,
                        lhsT=selP_sb[:, 128 * pair : 128 * pair + 128],
                        rhs=rzb,
                        start=True,
                        stop=True,
                    )
                    # walrus rejects TensorTensor with two PSUM operands:
                    # stage Mbc through SBUF, then stt (PSUM x SBUF).
                    mbc_sb = z_pool.tile([128, 512], bf16, tag="mbcs")
                    nc.vector.tensor_copy(out=mbc_sb, in_=mbc)
                    nc.vector.scalar_tensor_tensor(
                        out=yt_sb[:, pair, 512 * g : 512 * g + 512],
                        in0=ps_yg,
                        scalar=1.0,
                        in1=mbc_sb,
                        op0=Alu.mult,
                        op1=Alu.mult,
                    )
                while work:
                    work.pop(0)()

            def c0_pair(pair):
                return 512 * pair

            def projection_items(tq):
                o_holder = []

                def nh_pass(nh, last):
                    def run():
                        if not o_holder:
                            o_t_new = out_pool.tile([128, C], bf16, tag="o")
                            o_holder.append(o_t_new)
                        o_t = o_holder[0]
                        ps = ps_w_pool.tile([128, 512], fp32, tag="w")
                        for pair in range(2):
                            nc.tensor.matmul(
                                ps,
                                lhsT=yt_sb[:, pair, 128 * tq : 128 * tq + 128],
                                rhs=wp_sb[:, pair, 512 * nh : 512 * nh + 512],
                                start=(pair == 0),
                                stop=(pair == 1),
                            )
                        cp(o_t[:, 512 * nh : 512 * nh + 512], ps)
                        if last:
                            nc.sync.dma_start(
                                out=out[128 * tq : 128 * tq + 128, :],
                                in_=o_t,
                            )
                        del o_t

                    return run

                return [nh_pass(0, False), nh_pass(1, True)]

            # ---- schedule ----
            xt_tiles = [load_xt(c) for c in range(min(4, (T_ + 511) // 512))]
            for item in strip_items(0, xt_tiles[0]):
                item()
            for g in range(NG_):
                work = []
                if g > 0:
                    for t in range(4):
                        work += projection_items(4 * (g - 1) + t)
                if g + 1 < NG_:
                    work += strip_items(g + 1, xt_tiles[g + 1])
                pTs, zps = st_group(g, work)
                pv_norm_group(g, pTs, zps, work)
            for t in range(4):
                for item in projection_items(NQ_ - 4 + t):
                    item()

    nc.compile()
    return nc


def make_in_maps(x, W_attn, W_proj, T_=T):
    """Host-side sharding: per-core input dicts."""
    x = np.asarray(x, dtype=np.float32)[:, :T_, :]
    W_attn = np.asarray(W_attn, dtype=np.float32)
    W_proj = np.asarray(W_proj, dtype=np.float32)
    NQ_ = T_ // 128
    NA = max(1, NQ_ - 3)

    dvals = np.zeros(2 * T_ + 1024, dtype=np.float64)
    dvals[:T_] = _decay_values_np(T_)
    # ddSEQ4[k, a, r, q] = d(128(a + r) + q - k), 0 where negative
    k = np.arange(128)[:, None, None, None]
    a = np.arange(NA)[None, :, None, None]
    r = np.arange(4)[None, None, :, None]
    q = np.arange(128)[None, None, None, :]
    idx = 128 * (a + r) + q - k
    ddSEQ4 = np.where(idx >= 0, dvals[np.clip(idx, 0, idx.max())], 0.0).astype(BF16)
    ddSEQ4 = np.ascontiguousarray(ddSEQ4)

    tril4 = np.broadcast_to(
        (np.arange(128)[:, None] <= np.arange(128)[None, :])[:, None, :], (128, 4, 128)
    ).astype(np.float32).astype(BF16)
    tril4 = np.ascontiguousarray(tril4)
    ones32 = np.ones((128, 32), dtype=np.float32).astype(BF16)
    selP = np.zeros((128, 256), dtype=np.float32)
    for pair in range(2):
        for p in range(128):
            selP[64 * pair + 32 * (p // 64), 128 * pair + p] = 1.0
    selP = selP.astype(BF16)

    in_maps = []
    for core in range(N_CORES):
        b = core // 4
        g = core % 4
        h0 = HEADS_PER_CORE * g  # first head of this core within the batch
        xT_c = np.ascontiguousarray(x[b].T).astype(BF16)  # [C, T]
        cols = []
        for which in range(2):  # q, k
            base = 1024 * which
            for pair in range(2):
                h = h0 + 2 * pair
                cols.append(W_attn[:, base + 64 * h : base + 64 * (h + 2)])
        cols.append(W_attn[:, 2048 + 64 * h0 : 2048 + 64 * (h0 + 4)])  # v
        wqkv_c = np.concatenate(cols, axis=1).astype(BF16)  # [C, 768]
        wp_c = W_proj[64 * h0 : 64 * (h0 + 4), :].astype(BF16)  # [256, C]
        in_maps.append(
            {
                "xT": xT_c,
                "wqkv": wqkv_c,
                "wp": wp_c,
                "ddSEQ4": ddSEQ4,
                "tril4": tril4,
                "ones32": ones32,
                "selP": selP,
            }
        )
    return in_maps


def kernel(x, W_attn, W_proj):
    from concourse.bass_utils import run_bass_kernel_spmd

    in_maps = make_in_maps(x, W_attn, W_proj)
    nc = build_nc()
    res = run_bass_kernel_spmd(nc, in_maps, core_ids=list(range(N_CORES)))
    outs = [np.asarray(r["out"], dtype=np.float32) for r in res.results]
    full = np.zeros((B, T, C), dtype=np.float32)
    for core in range(N_CORES):
        full[core // 4] += outs[core]
    return full


# revision 23
# speedup vs baseline: 1.2228x; 1.2228x over previous
"""Causal self-attention with post-softmax decay mask — Trainium2 Bass kernel.

Problem shapes (hardcoded): B=2, T=2048, C=1024, H=16 heads, head_dim=64.
Sharding: 8 cores = (batch b = core//4) x (head group g = core%4, 4 heads each).
Each core computes QKV projection for its 4 heads from x[b], causal
flash-style attention with the decay mask, and a partial output projection
(its heads' 256 features x W_proj rows). Host sums the 4 partials per batch.

Key structure (v3 — transpose-free attention, groups of 4 q-tiles):
  The attention probabilities are computed directly in TRANSPOSED
  orientation, S^T[k, q] = (K Q^T), with k on partitions — so the PV
  contraction (over k) needs no XBAR transpose of P (the per-block
  transpose dispatch cost ~160ns x 544 blocks dominated v1), and the
  group-of-4 q-tile batching gives F=512 matmuls (per-matmul overhead
  dominated v2's F=256/group-2 variant).

  Per GROUP of 4 q-tiles (q columns 512g..512g+511), per head:
    - S^T k-blocks, kcd DESCENDING (jj=0 <-> kc=4g+3).  The 4 staircase
      blocks jj=0..3 are the q-tiles' diagonals: block jj only has valid
      q columns >= 128*(3-jj); the causal mask (-1e30) is an extra matmul
      negI^T @ maskA into each diagonal's 128-col region.  Head pairs
      pack into PE row halves (K=64, tile_position).
    - exp via ScalarE (scale=1/8) into SBUF pT [128, jj, tqi, 128].
    - Z row sums via PE: ones32^T @ pT per block (m=32, F<=512),
      accumulated into a 32-row band of one PSUM tile per head (4 heads
      = 4 distinct PE column groups; the bands cover the tile so the
      reciprocal reads no uninitialized rows).  Z uses the UNdecayed exp
      (the reference normalizes before the multiplicative decay).
    - decay: DVE/GpSimd tensor_tensor, ONE op for all full blocks via the
      contiguous Toeplitz constant ddSEQ4[k, a, r, q] = d(128(a+r)+q-k)
      (a = jj-3, r = tqi) + 3 small staircase ops.  d<0 positions are 0,
      doubling as causal zeroing.  Runs after the Z matmuls (in-place).
    - PV: F=512 matmuls over kc ascending (all 4 q-tiles at once; the
      staircase blocks shrink to F=384/256/128).
    - normalization: reciprocal_approx_fast of the banded Z tile,
      selector-matmul broadcast (selP, K=128) to head-halves, applied on
      the PSUM->SBUF copy (scalar_tensor_tensor).
  K/Q/V strips are produced per 512-t chunk from a double-buffered xT
  chunk tile; V^T strips are flipped to [t, feat] with one XBAR transpose
  per (chunk, 128-feat pass).  Projection per q-tile: 2 pair-accumulated
  F=512 matmuls per 512-feature half, one [128,1024] output DMA.
"""

import math
import sys

sys.path.insert(0, "/opt/trn_rl_repo")

import numpy as np
import ml_dtypes

B, T, C = 2, 2048, 1024
N_HEAD = 16
HD = 64
HEADS_PER_CORE = 4
N_CORES = 8
NQ = T // 128  # 16 q-tiles

BF16 = ml_dtypes.bfloat16


def _decay_values_np(n):
    """decay_values[i] = decay at distance i, faithful to reference (fp64)."""
    dl = 2048 - 16 + 1
    nums = np.linspace(0.0, 1.0, dl, dtype=np.float64)
    decay_values = 1.0 - np.power(nums, 1.0 / np.e)
    return np.concatenate([np.ones(15), decay_values])[:n]


def _decay_matrix_np(n):
    """tril decay matrix, faithful to reference.decay_weight_matrix (fp32)."""
    decay_values = _decay_values_np(n)
    idx = np.arange(n)[:, None] - np.arange(n)[None, :]
    mat = decay_values[np.clip(idx, 0, n - 1)]
    return np.where(idx >= 0, mat, 0.0).astype(np.float32)


def build_nc(T_=T):
    import concourse.bass as bass
    import concourse.bacc as bacc
    import concourse.mybir as mybir
    import concourse.tile as tile

    fp32 = mybir.dt.float32
    bf16 = mybir.dt.bfloat16
    Alu = mybir.AluOpType
    Act = mybir.ActivationFunctionType

    NQ_ = T_ // 128
    NG_ = NQ_ // 4  # q-tile groups of 4
    assert NQ_ % 4 == 0
    nc = bacc.Bacc("TRN2")

    xT = nc.declare_dram_parameter("xT", [C, T_], bf16, isOutput=False)
    # wqkv columns: [q01(128) q23(128) k01(128) k23(128) v0..v3(256)]
    wqkv = nc.declare_dram_parameter("wqkv", [C, 768], bf16, isOutput=False)
    # wp rows: h0 feats(64), h1, h2, h3
    wp = nc.declare_dram_parameter("wp", [256, C], bf16, isOutput=False)
    # Toeplitz decay arranged to match pT's (block, tqi, q) layout:
    # ddSEQ4[k, a, r, q] = d(128(a + r) + q - k), 0 where negative.
    NA = max(1, NQ_ - 3)
    ddSEQ4 = nc.declare_dram_parameter("ddSEQ4", [128, NA, 4, 128], bf16, isOutput=False)
    # causal tril mask applied on the DVE to the 4 diagonal sub-blocks
    # (post-exp, pre-Z): tril4[k, a, q] = 1 iff k <= q.
    tril4 = nc.declare_dram_parameter("tril4", [128, 4, 128], bf16, isOutput=False)
    # Z row-sum matmul weights (all ones, m=32 band)
    ones32 = nc.declare_dram_parameter("ones32", [128, 32], bf16, isOutput=False)
    # selector broadcast: selP[j, 128*pair + p] = 1 iff j = 64*pair + 32*(p//64)
    selP = nc.declare_dram_parameter("selP", [128, 256], bf16, isOutput=False)
    out = nc.declare_dram_parameter("out", [T_, C], bf16, isOutput=True)

    with tile.TileContext(nc) as tc:
        with (
            tc.tile_pool(name="const", bufs=1) as const_pool,
            tc.tile_pool(name="xt", bufs=4) as xt_pool,
            tc.tile_pool(name="qkvout", bufs=1) as qkv_pool,
            tc.tile_pool(name="p", bufs=1) as p_pool,
            tc.tile_pool(name="z", bufs=3) as z_pool,
            tc.tile_pool(name="outs", bufs=2) as out_pool,
            tc.tile_pool(name="ps_s", bufs=2, space="PSUM") as ps_s,
            tc.tile_pool(name="ps_w", bufs=2, space="PSUM") as ps_w_pool,
            tc.tile_pool(name="ps_y", bufs=1, space="PSUM") as ps_y_pool,
            tc.tile_pool(name="ps_z", bufs=1, space="PSUM") as ps_z_pool,
        ):
            # ---- constants ----
            wqkv_sb = const_pool.tile([128, 8, 768], bf16)
            wqkv_r = wqkv.rearrange("(kc p) m -> p kc m", p=128)
            for kc in range(8):
                nc.sync.dma_start(
                    out=wqkv_sb[:, kc : kc + 1, :], in_=wqkv_r[:, kc : kc + 1, :]
                )
            wp_sb = const_pool.tile([128, 2, C], bf16)
            nc.sync.dma_start(out=wp_sb, in_=wp.rearrange("(pr p) n -> p pr n", p=128))
            dd_sb = const_pool.tile([128, NA, 4, 128], bf16)
            nc.sync.dma_start(out=dd_sb, in_=ddSEQ4[:, :, :, :])
            tril4_sb = const_pool.tile([128, 4, 128], bf16)
            nc.sync.dma_start(out=tril4_sb, in_=tril4[:, :, :])
            ones32_sb = const_pool.tile([128, 32], bf16)
            nc.sync.dma_start(out=ones32_sb, in_=ones32[:, :])
            selP_sb = const_pool.tile([128, 256], bf16)
            nc.sync.dma_start(out=selP_sb, in_=selP[:, :])

            xT_r = xT.rearrange("(kc p) t -> p kc t", p=128)
            qt_sb = qkv_pool.tile([128, 2, T_], bf16)  # [feat(2x64), pair, q]
            kt_sb = qkv_pool.tile([128, 2, T_], bf16)
            vt_sb = qkv_pool.tile([128, 2, T_], bf16)  # v^T strips [feat pass, t]
            v_sb = qkv_pool.tile([128, T_ // 128, 256], bf16)  # [k-rows, kc, 4 heads]
            yt_sb = qkv_pool.tile([128, 2, T_], bf16)  # y^T strips per pair

            cp_flip = [0]

            def cp(dst, src):
                """PSUM->SBUF copies alternate Vector/Scalar."""
                e = nc.vector.tensor_copy if cp_flip[0] % 2 == 0 else nc.scalar.copy
                cp_flip[0] += 1
                e(out=dst, in_=src)

            # ---- phase 1: QKV strip production per 512-t chunk ----
            # emitted as individual work items (one strip pass each) so
            # they can be interleaved between S-piece bursts: the PE queue
            # is in-order, and these give it exp-independent work to chew
            # on while ScalarE catches up.
            def load_xt(c):
                t0 = 512 * c
                cw = min(512, T_ - t0)
                xt_c = xt_pool.tile([128, 8, 512], bf16, tag="xt")
                nc.sync.dma_start(
                    out=xt_c[:, :, 0:cw], in_=xT_r[:, :, t0 : t0 + cw]
                )
                return xt_c

            def strip_items(c, xt_c):
                t0 = 512 * c
                cw = min(512, T_ - t0)

                def qk_pass(which, dst, pair):
                    def run():
                        wcol = 256 * which + 128 * pair
                        ps = ps_w_pool.tile([128, 512], fp32, tag="w")
                        for kc in range(8):
                            nc.tensor.matmul(
                                ps[:, 0:cw],
                                lhsT=wqkv_sb[:, kc, wcol : wcol + 128],
                                rhs=xt_c[:, kc, 0:cw],
                                start=(kc == 0),
                                stop=(kc == 7),
                            )
                        cp(dst[:, pair, t0 : t0 + cw], ps[:, 0:cw])

                    return run

                def v_pass(p):
                    def run():
                        ps = ps_w_pool.tile([128, 512], fp32, tag="w")
                        for kc in range(8):
                            nc.tensor.matmul(
                                ps[:, 0:cw],
                                lhsT=wqkv_sb[:, kc, 512 + 128 * p : 640 + 128 * p],
                                rhs=xt_c[:, kc, 0:cw],
                                start=(kc == 0),
                                stop=(kc == 7),
                            )
                        cp(vt_sb[:, p, t0 : t0 + cw], ps[:, 0:cw])
                        nc.sync.dma_start_transpose(
                            out=v_sb[
                                :, 4 * c : 4 * c + cw // 128, 128 * p : 128 * p + 128
                            ],
                            in_=vt_sb[:, p, t0 : t0 + cw],
                        )

                    return run

                return [
                    qk_pass(1, kt_sb, 0),
                    qk_pass(1, kt_sb, 1),
                    qk_pass(0, qt_sb, 0),
                    qk_pass(0, qt_sb, 1),
                    v_pass(0),
                    v_pass(1),
                ]

            # ---- phase 2: S^T / exp / Z / decay per group of 4 q-tiles ----
            def st_group(g, work):
                nblk = 4 * g + 4  # k-blocks, jj descending: jj=0 <-> kc=4g+3
                npiece = nblk // 2
                pTs = []
                for head in range(4):
                    # [k, jj, tqi, q]
                    pT = p_pool.tile([128, NQ_, 4, 128], bf16, tag=f"p{head}")
                    pTs.append(pT)
                zps = ps_z_pool.tile([128, 512], fp32, tag="z")
                q0 = 512 * g
                for piece in range(npiece):
                    j0 = 2 * piece
                    for head in range(4):
                        pair, hin = head // 2, head % 2
                        prow = 64 * hin
                        pT = pTs[head]
                        pTf = pT[:, :, :, :].rearrange("p a b q -> p (a b q)")
                        ps_full = ps_s.tile([128, 1024], fp32, tag="s")
                        ps = ps_full
                        for jj in (j0, j0 + 1):
                            kc = 4 * g + 3 - jj
                            c0 = 512 * (jj - j0)
                            lo = 128 * max(0, 3 - jj)
                            nc.tensor.matmul(
                                ps[:, c0 + lo : c0 + 512],
                                lhsT=kt_sb[
                                    prow : prow + 64, pair, 128 * kc : 128 * kc + 128
                                ],
                                rhs=qt_sb[prow : prow + 64, pair, q0 + lo : q0 + 512],
                                start=True,
                                stop=True,
                                tile_position=(prow, 0),
                            )
                        if piece == 0:
                            nc.scalar.activation(
                                out=pTf[:, 384:512],
                                in_=ps[:, 384:512],
                                func=Act.Exp,
                                scale=0.125,
                            )
                            nc.scalar.activation(
                                out=pTf[:, 768:1024],
                                in_=ps[:, 768:1024],
                                func=Act.Exp,
                                scale=0.125,
                            )
                        elif piece == 1:
                            nc.scalar.activation(
                                out=pTf[:, 1152:2048],
                                in_=ps[:, 128:1024],
                                func=Act.Exp,
                                scale=0.125,
                            )
                        else:
                            nc.scalar.activation(
                                out=pTf[:, 512 * j0 : 512 * (j0 + 2)],
                                in_=ps[:, 0:1024],
                                func=Act.Exp,
                                scale=0.125,
                            )
                    # exp-independent PE filler between S-piece bursts
                    if len(work) > 2:
                        work.pop(0)()
                # causal tril zeroing of the 4 diagonal sub-blocks (flat
                # cols 384 + 384*jj .. +128), one strided DVE op per head
                for head in range(4):
                    eng = nc.gpsimd if head == 3 else nc.vector
                    pTf = pTs[head][:, :, :, :].rearrange("p a b q -> p (a b q)")
                    diag = pTf[:, 384:1920].rearrange("p (a q) -> p a q", q=384)[
                        :, :, 0:128
                    ]
                    eng.tensor_tensor(
                        out=diag, in0=diag, in1=tril4_sb, op=Alu.mult
                    )
                # Z row sums (undecayed), kc ascending so the full-width
                # block opens the accumulation group; head-inner so the 4
                # column-group bands run concurrently on the PE
                for jj in range(nblk - 1, -1, -1):
                    lo3 = max(0, 3 - jj)
                    for head in range(4):
                        nc.tensor.matmul(
                            zps[32 * head : 32 * head + 32, 128 * lo3 : 512],
                            lhsT=ones32_sb,
                            rhs=pTs[head][:, jj, lo3:4, :],
                            start=(jj == nblk - 1),
                            stop=(jj == 0),
                            tile_position=(0, 32 * head),
                            skip_group_check=True,
                        )
                # decay (in place, after the Z sums): one op for the full
                # blocks + three staircase ops
                for head in range(4):
                    eng = nc.gpsimd if head == 3 else nc.vector
                    pT = pTs[head]
                    hi = nblk
                    while hi > 3:
                        lo_j = max(3, hi - 4)
                        eng.tensor_tensor(
                            out=pT[:, lo_j:hi, :, :],
                            in0=pT[:, lo_j:hi, :, :],
                            in1=dd_sb[:, lo_j - 3 : hi - 3, :, :],
                            op=Alu.mult,
                        )
                        hi = lo_j
                    eng.tensor_tensor(
                        out=pT[:, 2, 1:4, :],
                        in0=pT[:, 2, 1:4, :],
                        in1=dd_sb[:, 0, 0:3, :],
                        op=Alu.mult,
                    )
                    eng.tensor_tensor(
                        out=pT[:, 1, 2:4, :],
                        in0=pT[:, 1, 2:4, :],
                        in1=dd_sb[:, 0, 0:2, :],
                        op=Alu.mult,
                    )
                    eng.tensor_tensor(
                        out=pT[:, 0, 3, :],
                        in0=pT[:, 0, 3, :],
                        in1=dd_sb[:, 0, 0, :],
                        op=Alu.mult,
                    )
                return pTs, zps

            # ---- phase 3: grouped PV + deferred softmax normalization ----
            def pv_norm_group(g, pTs, zps, work):
                nblk = 4 * g + 4
                # drain remaining filler here: it sits in the PE queue at
                # the decay-wait point, before the PV chains
                while work:
                    work.pop(0)()
                rz = z_pool.tile([128, 512], fp32, tag="rz")
                nc.vector.reciprocal_approx_fast(out=rz, in_=zps)
                rzb = z_pool.tile([128, 512], bf16, tag="rzb")
                nc.vector.tensor_copy(out=rzb, in_=rz)
                for pair in range(2):
                    ps_yg = ps_y_pool.tile([128, 512], fp32, tag="y")
                    for kc in range(nblk):
                        jj = 4 * g + 3 - kc
                        lo3 = max(0, 3 - jj)
                        for hin in range(2):
                            head = 2 * pair + hin
                            prow = 64 * hin
                            nc.tensor.matmul(
                                ps_yg[prow : prow + 64, 128 * lo3 : 512],
                                lhsT=v_sb[:, kc, 64 * head : 64 * head + 64],
                                rhs=pTs[head][:, jj, lo3:4, :],
                                start=(kc == 0),
                                stop=(kc == nblk - 1),
                                tile_position=(0, prow),
                                skip_group_check=True,
                            )
                    mbc = ps_w_pool.tile([128, 512], fp32, tag="w")
                    nc.tensor.matmul(
                        mbc,
                        lhsT=selP_sb[:, 128 * pair : 128 * pair + 128],
                        rhs=rzb,
                        start=True,
                        stop=True,
                    )
                    # walrus rejects TensorTensor with two PSUM operands:
                    # stage Mbc through SBUF, then stt (PSUM x SBUF).
                    mbc_sb = z_pool.tile([128, 512], bf16, tag="mbcs")
                    nc.vector.tensor_copy(out=mbc_sb, in_=mbc)
                    nc.vector.scalar_tensor_tensor(
                        out=yt_sb[:, pair, 512 * g : 512 * g + 512],
                        in0=ps_yg,
                        scalar=1.0,
                        in1=mbc_sb,
                        op0=Alu.mult,
                        op1=Alu.mult,
                    )
                while work:
                    work.pop(0)()

            def c0_pair(pair):
                return 512 * pair

            def projection_items(tq):
                o_holder = []

                def nh_pass(nh, last):
                    def run():
                        if not o_holder:
                            o_t_new = out_pool.tile([128, C], bf16, tag="o")
                            o_holder.append(o_t_new)
                        o_t = o_holder[0]
                        ps = ps_w_pool.tile([128, 512], fp32, tag="w")
                        for pair in range(2):
                            nc.tensor.matmul(
                                ps,
                                lhsT=yt_sb[:, pair, 128 * tq : 128 * tq + 128],
                                rhs=wp_sb[:, pair, 512 * nh : 512 * nh + 512],
                                start=(pair == 0),
                                stop=(pair == 1),
                            )
                        cp(o_t[:, 512 * nh : 512 * nh + 512], ps)
                        if last:
                            nc.sync.dma_start(
                                out=out[128 * tq : 128 * tq + 128, :],
                                in_=o_t,
                            )
                        del o_t

                    return run

                return [nh_pass(0, False), nh_pass(1, True)]

            # ---- schedule ----
            xt_tiles = [load_xt(c) for c in range(min(4, (T_ + 511) // 512))]
            for item in strip_items(0, xt_tiles[0]):
                item()
            for g in range(NG_):
                work = []
                if g > 0:
                    for t in range(4):
                        work += projection_items(4 * (g - 1) + t)
                if g + 1 < NG_:
                    work += strip_items(g + 1, xt_tiles[g + 1])
                pTs, zps = st_group(g, work)
                pv_norm_group(g, pTs, zps, work)
            for t in range(4):
                for item in projection_items(NQ_ - 4 + t):
                    item()

    nc.compile()
    return nc


def make_in_maps(x, W_attn, W_proj, T_=T):
    """Host-side sharding: per-core input dicts."""
    x = np.asarray(x, dtype=np.float32)[:, :T_, :]
    W_attn = np.asarray(W_attn, dtype=np.float32)
    W_proj = np.asarray(W_proj, dtype=np.float32)
    NQ_ = T_ // 128
    NA = max(1, NQ_ - 3)

    dvals = np.zeros(2 * T_ + 1024, dtype=np.float64)
    dvals[:T_] = _decay_values_np(T_)
    # ddSEQ4[k, a, r, q] = d(128(a + r) + q - k), 0 where negative
    k = np.arange(128)[:, None, None, None]
    a = np.arange(NA)[None, :, None, None]
    r = np.arange(4)[None, None, :, None]
    q = np.arange(128)[None, None, None, :]
    idx = 128 * (a + r) + q - k
    ddSEQ4 = np.where(idx >= 0, dvals[np.clip(idx, 0, idx.max())], 0.0).astype(BF16)
    ddSEQ4 = np.ascontiguousarray(ddSEQ4)

    tril4 = np.broadcast_to(
        (np.arange(128)[:, None] <= np.arange(128)[None, :])[:, None, :], (128, 4, 128)
    ).astype(np.float32).astype(BF16)
    tril4 = np.ascontiguousarray(tril4)
    ones32 = np.ones((128, 32), dtype=np.float32).astype(BF16)
    selP = np.zeros((128, 256), dtype=np.float32)
    for pair in range(2):
        for p in range(128):
            selP[64 * pair + 32 * (p // 64), 128 * pair + p] = 1.0
    selP = selP.astype(BF16)

    in_maps = []
    for core in range(N_CORES):
        b = core // 4
        g = core % 4
        h0 = HEADS_PER_CORE * g  # first head of this core within the batch
        xT_c = np.ascontiguousarray(x[b].T).astype(BF16)  # [C, T]
        cols = []
        for which in range(2):  # q, k
            base = 1024 * which
            for pair in range(2):
                h = h0 + 2 * pair
                cols.append(W_attn[:, base + 64 * h : base + 64 * (h + 2)])
        cols.append(W_attn[:, 2048 + 64 * h0 : 2048 + 64 * (h0 + 4)])  # v
        wqkv_c = np.concatenate(cols, axis=1).astype(BF16)  # [C, 768]
        wp_c = W_proj[64 * h0 : 64 * (h0 + 4), :].astype(BF16)  # [256, C]
        in_maps.append(
            {
                "xT": xT_c,
                "wqkv": wqkv_c,
                "wp": wp_c,
                "ddSEQ4": ddSEQ4,
                "tril4": tril4,
                "ones32": ones32,
                "selP": selP,
            }
        )
    return in_maps


def kernel(x, W_attn, W_proj):
    from concourse.bass_utils import run_bass_kernel_spmd

    in_maps = make_in_maps(x, W_attn, W_proj)
    nc = build_nc()
    res = run_bass_kernel_spmd(nc, in_maps, core_ids=list(range(N_CORES)))
    outs = [np.asarray(r["out"], dtype=np.float32) for r in res.results]
    full = np.zeros((B, T, C), dtype=np.float32)
    for core in range(N_CORES):
        full[core // 4] += outs[core]
    return full
